# revision 30
# baseline (speedup 1.0000x reference)
"""Trainium2 Bass kernel for an attention block (B=8, T=2048, D=K=V=1024).

Reference math (per batch element, sharded one per NeuronCore):
    Q = x @ Wq.T + bq ; K = x @ Wk.T + bk ; V = x @ Wv.T + bv
    logits[t,s] = Q[t] . K[s],  masked -inf for s > t (strict upper tri)
    probs = softmax(logits, axis=t) / sqrt(1024)     # softmax over QUERY axis
    out = x + probs @ V

v4: every matmul runs fp8 (e4m3) DoubleRow (256-deep contraction per
instruction, ~141 TF/s measured). Transposes run on the PE in fp8 (inputs are
converted fp32->fp8 once, weights pre-scaled x32 into e4m3's range), batched
4-per-PSUM-tile so each [128,512] PSUM drain is one vector/scalar copy.
  - QT/KT epilogue (psum/32 + bias -> fp8) is one fused tensor_scalar (vector)
    or activation (scalar), alternating.
  - P = exp(logits) kept in SBUF bf16 (no max subtraction: |logits| < ~80).
  - Softmax over t: Z[s] accumulates via exp's accum_out; during the last
    t-block's sweep R[s]=1/Z finalizes per s-tile, P columns convert to fp8
    (P*R in [0,1]); PV matmuls + residual epilogue run two steps behind so
    the exp->R->convert chain stays off the PE critical path.
  - Vp = 32(V+bv) fp8; epilogue: out = psum/1024 + x_bf16 (residual kept in
    SBUF from the front pass; bf16 residual adds ~0.2% err vs 2e-2 budget).
  - Odd PV contractions round up to a DoubleRow pair: the extra s-block is
    always fully-masked (exp == 0), contributing nothing.
Measured numerics: rel_err ~4.6e-3 (tolerance 2e-2).
"""

import time

import numpy as np

import concourse.bass as bass
import concourse.bacc as bacc
import concourse.mybir as mybir
import concourse.tile as tile
from concourse.bass_utils import run_bass_kernel_spmd
from concourse.masks import make_identity

F32 = mybir.dt.float32
BF16 = mybir.dt.bfloat16
FP8 = mybir.dt.float8e4
AF = mybir.ActivationFunctionType
DR = mybir.MatmulPerfMode.DoubleRow
MULT = mybir.AluOpType.mult
ADD = mybir.AluOpType.add

P = 128          # partitions
T = 2048         # sequence length
D = 1024         # model dim
TB = 512         # t-block width
NTB = 4          # t-blocks
KO = 8           # k output tiles of 128
DK = 8           # contraction subtiles of 128
SV = 16          # s tiles of 128
NEG = -1.0e30
WS = 32.0        # weight quantization scale
PBASE = [0, 4, 12]   # flat index base of j<3 causal P tiles in Pbig


def _build_nc():
    nc = bacc.Bacc("TRN2", target_bir_lowering=False, debug=False, num_devices=8)

    x = nc.dram_tensor("x", [T, D], F32, kind="ExternalInput").ap()
    Wq = nc.dram_tensor("Wq", [D, D], F32, kind="ExternalInput").ap()
    bq = nc.dram_tensor("bq", [D], F32, kind="ExternalInput").ap()
    Wk = nc.dram_tensor("Wk", [D, D], F32, kind="ExternalInput").ap()
    bk = nc.dram_tensor("bk", [D], F32, kind="ExternalInput").ap()
    Wv = nc.dram_tensor("Wv", [D, D], F32, kind="ExternalInput").ap()
    bv = nc.dram_tensor("bv", [D], F32, kind="ExternalInput").ap()
    out = nc.dram_tensor("out", [T, D], F32, kind="ExternalOutput").ap()

    with tile.TileContext(nc) as tc:
        _kernel_body(nc, tc, x, Wq, bq, Wk, bk, Wv, bv, out)

    nc.compile()
    return nc


def _kernel_body(nc, tc, x, Wq, bq, Wk, bk, Wv, bv, out):
    from contextlib import ExitStack

    ctx = ExitStack()
    with ctx:
        consts = ctx.enter_context(tc.tile_pool(name="consts", bufs=1))
        wt8p = ctx.enter_context(tc.tile_pool(name="wt8", bufs=1))
        xt8p = ctx.enter_context(tc.tile_pool(name="xt8", bufs=1))
        kt8p = ctx.enter_context(tc.tile_pool(name="kt8", bufs=1))
        vp8p = ctx.enter_context(tc.tile_pool(name="vp8", bufs=1))
        pbigp = ctx.enter_context(tc.tile_pool(name="pbig", bufs=1))
        pq8p = ctx.enter_context(tc.tile_pool(name="pq8", bufs=1))
        qt8p = ctx.enter_context(tc.tile_pool(name="qt8", bufs=2))
        xbp = ctx.enter_context(tc.tile_pool(name="xb", bufs=1))
        pstp = ctx.enter_context(tc.tile_pool(name="pst", bufs=3))
        natp = ctx.enter_context(tc.tile_pool(name="nat", bufs=4))
        cv8p = ctx.enter_context(tc.tile_pool(name="cv8", bufs=3))
        ostp = ctx.enter_context(tc.tile_pool(name="ost", bufs=2))
        psum_t = ctx.enter_context(tc.tile_pool(name="psum_t", bufs=2, space="PSUM"))
        psum_mm = ctx.enter_context(tc.tile_pool(name="psum_mm", bufs=6, space="PSUM"))

        # ---- identity first: it gates every PE transpose at kernel start ----
        id32 = consts.tile([P, P], F32, name="id32")
        make_identity(nc, id32)
        idb = consts.tile([P, P], BF16, name="idb")
        nc.vector.tensor_copy(out=idb, in_=id32)
        # identity x1024: adds the residual into the PV psum (which holds
        # 1024*read) via one matmul; 1024 is a power of two so exact in bf16
        id1k = consts.tile([P, P], BF16, name="id1k")
        nc.vector.tensor_scalar_mul(id1k, id32, WS * WS)

        # persistent fp8 operand tensors
        WqT8 = wt8p.tile([P, DK, D], FP8, name="WqT8")   # (32 Wq)^T [d_in, dk, k]
        WkT8 = wt8p.tile([P, DK, D], FP8, name="WkT8")
        WvT8 = wt8p.tile([P, DK, D], FP8, name="WvT8")
        xT8 = xt8p.tile([P, DK, T], FP8, name="xT8")     # x^T [d_in, dk, t]
        KT8 = kt8p.tile([P, KO, T], FP8, name="KT8")     # (K+bk)^T [k_in, ko, s]
        Vp8 = vp8p.tile([P, SV, D], FP8, name="Vp8")     # 32(V+bv) [s_in, sv, v]
        Pbig = pbigp.tile([P, 24, TB], BF16, name="Pbig")  # exp(logits), j<3
        Pq8 = [pq8p.tile([P, 4 * j + 4, TB], FP8, name=f"Pq8_{j}")
               for j in range(NTB)]                      # P/Z [s_in, sv, t] per j
        xb = xbp.tile([P, SV, D], BF16, name="xb")       # x rows bf16 (residual)

        def dma_in_split(dst, src, nsplit=2):
            step = P // nsplit
            for q in range(nsplit):
                eng = nc.gpsimd if q % 2 == 0 else nc.sync
                eng.dma_start(out=dst[q * step:(q + 1) * step, :],
                              in_=src[q * step:(q + 1) * step, :])

        def transpose_group(srcb, dst, dk0, dst_col, grp):
            """4 bf16 PE transposes into one [128,512] psum tile, 1 fp8 drain."""
            pt = psum_t.tile([P, 4 * P], BF16, name="pt", tag="pt")
            for q in range(4):
                dk = dk0 + q
                nc.tensor.transpose(
                    pt[:, q * P:(q + 1) * P],
                    srcb[:, dk * P:(dk + 1) * P], idb)
            # dst slice [128, 4(dk), 128] <- pt [128, 4*128]
            dview = dst[:, dk0:dk0 + 4, dst_col:dst_col + P]
            pview = pt.rearrange("p (g c) -> p g c", g=4)
            if grp % 2 == 0:
                nc.vector.tensor_copy(out=dview, in_=pview)
            else:
                nc.scalar.activation(dview, pview, AF.Copy)

        grp_ctr = [0]

        def xb_copy(ti, xnat):
            # two half-copies on parallel engines: each transpose group's
            # 512-column source lands ~0.5us sooner than one wide copy
            nc.vector.tensor_copy(out=xb[:, ti, 0:TB], in_=xnat[:, 0:TB])
            nc.scalar.activation(xb[:, ti, TB:D], xnat[:, TB:D], AF.Copy)

        def emit_x_block(j):
            """DMA x rows, bf16 copy (residual + transpose src), -> xT8 fp8."""
            for ts_ in range(TB // P):
                ti = 4 * j + ts_
                t0 = ti * P
                xnat = natp.tile([P, D], F32, name="xnat", tag="nat")
                dma_in_split(xnat, x[t0:t0 + P, :], nsplit=4 if j == 0 else 2)
                xb_copy(ti, xnat)
                for dk0 in (0, 4):
                    transpose_group(xb[:, ti, :], xT8, dk0, t0, grp_ctr[0])
                    grp_ctr[0] += 1

        def emit_w_tile(w_ap, dst, kt, wnat=None):
            """W row-tile kt (preloaded or DMA'd), x32 bf16, transpose."""
            if wnat is None:
                wnat = natp.tile([P, D], F32, name="wnat", tag="nat")
                dma_in_split(wnat, w_ap[kt * P:(kt + 1) * P, :])
            wb = cv8p.tile([P, D], BF16, name="wb", tag="cv8")
            if kt % 2 == 0:
                nc.vector.tensor_scalar_mul(wb, wnat, WS)
            else:
                nc.scalar.activation(wb, wnat, AF.Copy, scale=WS)
            for dk0 in (0, 4):
                transpose_group(wb, dst, dk0, kt * P, grp_ctr[0])
                grp_ctr[0] += 1

        # x(j0) tiles interleaved with early Wq row-tile DMAs: x tile 0 stays
        # first (it gates the first PE transpose), Wq streams right behind
        wq_pre = []

        def emit_x_tile(ti, nsplit=2):
            t0 = ti * P
            xnat = natp.tile([P, D], F32, name="xnat", tag="nat")
            dma_in_split(xnat, x[t0:t0 + P, :], nsplit=nsplit)
            xb_copy(ti, xnat)
            for dk0 in (0, 4):
                transpose_group(xb[:, ti, :], xT8, dk0, t0, grp_ctr[0])
                grp_ctr[0] += 1

        for ti in range(4):
            emit_x_tile(ti, nsplit=4)
            wnat = natp.tile([P, D], F32, name="wnat", tag="nat")
            dma_in_split(wnat, Wq[ti * P:(ti + 1) * P, :])
            wq_pre.append(wnat)

        # sliding staircase mask [128, 896]: 0 iff free >= part + 384 else -1e30
        mask_base = consts.tile([P, TB + 3 * P], BF16, name="mask_base")
        nc.gpsimd.memset(mask_base, 0.0)
        nc.gpsimd.affine_select(
            out=mask_base, in_=mask_base,
            compare_op=mybir.AluOpType.is_ge,
            fill=NEG,
            base=-(3 * P),
            pattern=[[1, TB + 3 * P]],
            channel_multiplier=-1,
        )
        masks = [mask_base[:, 3 * P - oi * P: 3 * P - oi * P + TB]
                 for oi in range(4)]

        # biases: bq/bk striped [128, 8]; bv broadcast + x32 (matches Vp8)
        bq_sb = consts.tile([P, KO], F32, name="bq_sb")
        nc.sync.dma_start(out=bq_sb, in_=bq.rearrange("(o p) -> p o", p=P))
        bk_sb = consts.tile([P, KO], F32, name="bk_sb")
        nc.sync.dma_start(out=bk_sb, in_=bk.rearrange("(o p) -> p o", p=P))
        bv_sb = consts.tile([P, D], F32, name="bv_sb")
        bv_bcast = bass.AP(tensor=bv.tensor, offset=bv.offset,
                           ap=[[0, P], [1, D]])
        nc.gpsimd.dma_start(out=bv_sb, in_=bv_bcast)
        bv32_sb = consts.tile([P, D], BF16, name="bv32_sb")
        nc.scalar.activation(bv32_sb, bv_sb, AF.Copy, scale=WS)

        Zacc = consts.tile([P, SV, NTB], F32, name="Zacc")
        nc.vector.memset(Zacc, 0.0)
        ztmp = consts.tile([P, SV], F32, name="ztmp")
        rtile = consts.tile([P, SV], F32, name="rtile")

        def emit_qkt_ko(j, wt8, bias_sb, dst, ko):
            """One QT/KT column tile [k 128, t 512], fused epilogue -> fp8."""
            ps = psum_mm.tile([P, TB], F32, name="ps_qk", tag="mm")
            for a in range(4):
                nc.tensor.matmul(
                    ps,
                    lhsT=wt8[:, 2 * a:2 * a + 2, ko * P:(ko + 1) * P],
                    rhs=xT8[:, 2 * a:2 * a + 2, j * TB:(j + 1) * TB],
                    start=(a == 0), stop=(a == 3),
                    perf_mode=DR,
                )
            if ko % 2 == 0:
                nc.vector.tensor_scalar(
                    out=dst, in0=ps,
                    scalar1=1.0 / WS, scalar2=bias_sb[:, ko:ko + 1],
                    op0=MULT, op1=ADD,
                )
            else:
                nc.scalar.activation(
                    dst, ps, AF.Identity,
                    bias=bias_sb[:, ko:ko + 1], scale=1.0 / WS,
                )

        def emit_v_unit(j, si, h):
            """One Vp8 tile [s 128, v 512] = 32(V+bv) fp8."""
            sv = 4 * j + si
            s0 = sv * P
            ps = psum_mm.tile([P, TB], F32, name="ps_v", tag="mm")
            for a in range(4):
                nc.tensor.matmul(
                    ps,
                    lhsT=xT8[:, 2 * a:2 * a + 2, s0:s0 + P],
                    rhs=WvT8[:, 2 * a:2 * a + 2, h * TB:(h + 1) * TB],
                    start=(a == 0), stop=(a == 3),
                    perf_mode=DR,
                )
            nc.vector.tensor_add(
                out=Vp8[:, sv, h * TB:(h + 1) * TB],
                in0=ps, in1=bv32_sb[:, h * TB:(h + 1) * TB],
            )

        def emit_logits_exp(j, sv, qt8):
            """logits tile [s 128, t 512] -> exp -> Pbig/pst; Z accum."""
            ps = psum_mm.tile([P, TB], F32, name="ps_l", tag="mm")
            for a in range(4):
                nc.tensor.matmul(
                    ps,
                    lhsT=KT8[:, 2 * a:2 * a + 2, sv * P:(sv + 1) * P],
                    rhs=qt8[:, 2 * a:2 * a + 2, :],
                    start=(a == 0), stop=(a == 3),
                    perf_mode=DR,
                )
            oi = sv - 4 * j
            if oi >= 0:
                nc.vector.tensor_add(out=ps, in0=ps, in1=masks[oi])
            if j < NTB - 1:
                dst = Pbig[:, PBASE[j] + sv, :]
            else:
                dst = pstp.tile([P, TB], BF16, name="pst", tag="pst")
            nc.scalar.activation(
                dst, ps, AF.Exp, accum_out=Zacc[:, sv, j:j + 1],
            )
            return dst

        def emit_out_tile(i):
            """out rows [i*128, (i+1)*128): PV fp8 DR; residual added in-PSUM
            via one id1k matmul (psum holds 1024*(read + x)); 1-op epilogue."""
            jj = i // 4
            tc_ = i % 4
            ost = ostp.tile([P, D], F32, name="ost", tag="ost")
            npair = (i + 2) // 2
            for h in range(D // TB):
                ps = psum_mm.tile([P, TB], F32, name="ps_o", tag="mm")
                for a in range(npair):
                    nc.tensor.matmul(
                        ps,
                        lhsT=Pq8[jj][:, 2 * a:2 * a + 2, tc_ * P:(tc_ + 1) * P],
                        rhs=Vp8[:, 2 * a:2 * a + 2, h * TB:(h + 1) * TB],
                        start=(a == 0), stop=False,
                        perf_mode=DR,
                    )
                nc.tensor.matmul(
                    ps, lhsT=id1k, rhs=xb[:, i, h * TB:(h + 1) * TB],
                    start=False, stop=True,
                )
                oh = ost[:, h * TB:(h + 1) * TB]
                if h == 0:
                    nc.scalar.activation(oh, ps, AF.Copy, scale=1.0 / (WS * WS))
                else:
                    nc.vector.tensor_scalar_mul(oh, ps, 1.0 / (WS * WS))
                nc.sync.dma_start(
                    out=out[i * P:(i + 1) * P, h * TB:(h + 1) * TB], in_=oh)

        # ---- main pipeline ----
        for j in range(NTB):
            if j > 0:
                emit_x_block(j)
            qt8 = qt8p.tile([P, KO, TB], FP8, name="qt8", tag="qt8")
            if j == 0:
                # interleave each W row-tile with the matmuls it unblocks:
                # W tile kt provides exactly the ko=kt stationary columns
                for kt in range(8):
                    emit_w_tile(Wq, WqT8, kt,
                                wnat=wq_pre[kt] if kt < 4 else None)
                    emit_qkt_ko(0, WqT8, bq_sb, qt8[:, kt, :], kt)
                for kt in range(8):
                    emit_w_tile(Wk, WkT8, kt)
                    emit_qkt_ko(0, WkT8, bk_sb, KT8[:, kt, 0:TB], kt)
                for kt in range(8):
                    emit_w_tile(Wv, WvT8, kt)
                    if kt == 3:
                        for si in range(4):
                            emit_v_unit(0, si, 0)
                for si in range(4):
                    emit_v_unit(0, si, 1)
            else:
                for ko in range(KO):
                    emit_qkt_ko(j, WqT8, bq_sb, qt8[:, ko, :], ko)
                for ko in range(KO):
                    emit_qkt_ko(j, WkT8, bk_sb,
                                KT8[:, ko, j * TB:(j + 1) * TB], ko)
                for si in range(TB // P):
                    for h in range(D // TB):
                        emit_v_unit(j, si, h)

            for sv in range(4 * (j + 1)):
                pdst = emit_logits_exp(j, sv, qt8)
                if j == NTB - 1:
                    # Z[sv] final: R = 1/Z; normalize+convert column sv of
                    # every j' block to fp8; out-tiles lag 2 steps so the
                    # exp->R->convert chain stays off the PE critical path
                    nc.vector.reduce_sum(out=ztmp[:, sv:sv + 1],
                                         in_=Zacc[:, sv, :],
                                         axis=mybir.AxisListType.X)
                    nc.vector.reciprocal(rtile[:, sv:sv + 1],
                                         ztmp[:, sv:sv + 1])
                    for jp in range(NTB):
                        if sv > 4 * jp + 3:
                            continue
                        src = pdst if jp == 3 else Pbig[:, PBASE[jp] + sv, :]
                        if (jp + sv) % 2 == 0:
                            nc.vector.tensor_scalar_mul(
                                Pq8[jp][:, sv, :], src, rtile[:, sv:sv + 1])
                        else:
                            nc.scalar.activation(
                                Pq8[jp][:, sv, :], src,
                                AF.Identity, scale=rtile[:, sv:sv + 1])
                    if sv >= 2:
                        emit_out_tile(sv - 2)
        emit_out_tile(SV - 2)
        emit_out_tile(SV - 1)


_NC_CACHE = None


def _get_nc():
    global _NC_CACHE
    if _NC_CACHE is None:
        _NC_CACHE = _build_nc()
    return _NC_CACHE


def kernel(minibatch, Wq, bq, Wk, bk, Wv, bv):
    minibatch = np.asarray(minibatch, dtype=np.float32)
    Wq = np.asarray(Wq, dtype=np.float32)
    bq = np.asarray(bq, dtype=np.float32)
    Wk = np.asarray(Wk, dtype=np.float32)
    bk = np.asarray(bk, dtype=np.float32)
    Wv = np.asarray(Wv, dtype=np.float32)
    bv = np.asarray(bv, dtype=np.float32)

    nc = _get_nc()
    B = minibatch.shape[0]
    in_maps = [
        {
            "x": np.ascontiguousarray(minibatch[i]),
            "Wq": Wq, "bq": bq, "Wk": Wk, "bk": bk, "Wv": Wv, "bv": bv,
        }
        for i in range(B)
    ]
    last_err = None
    for _attempt in range(3):
        try:
            res = run_bass_kernel_spmd(nc, in_maps, core_ids=list(range(B)))
            break
        except Exception as e:  # transient device errors
            last_err = e
            time.sleep(2.0)
    else:
        raise last_err
    return np.stack([res.results[i]["out"] for i in range(B)], axis=0)
